# revision 7
# baseline (speedup 1.0000x reference)
"""MoE (8 experts, top-2, d=1024, N=8192) on 8 trn2 NeuronCores.

Strategy (expert-parallel, per sharding hint):
 - Host computes routing (top-2 expert ids per token, fp64 logits) and the
   top-2 softmax gates, then dispatches WEIGHTED tokens: the gate is folded
   into the dispatched activation row (g*x), so the device kernel is a pure
   GEMM — "combine with weighted all-to-all back" done at dispatch time.
 - Load balancing: every core gets exactly T = T0+1 token tiles of 128.
   Tiles 0..T0-1 hold (up to) T0*128 tokens of the core's own expert
   (weight slot w). Tile T0 is an overflow tile: spill tokens from experts
   whose load exceeds T0*128 are redistributed across cores, each core
   receiving one <=128-token chunk with that expert's weights in slot w2.
   For the common near-balanced case this gives T=17 tiles/core instead of
   the 18 a max-load-padded per-expert split would need.
 - Device (per core, SPMD): for each 128-token tile: 16 matmuls (8 K-chunks
   x 2 psum halves of 512, j-outer so both halves share one LDWEIGHTS) in
   bf16 (1 cycle/row, half the DMA bytes of fp32), PSUM f32 accumulate,
   evacuate PSUM->SBUF with a bf16 downcast alternating Vector/Scalar
   engines, DMA out. Input tiles are DMA'd in groups of G for fewer, larger
   descriptors; groups are software-pipelined (load i+1 before compute i).
 - Host combines: out[idx] += y (+ g*b[e]); each token appears in exactly
   two experts' lists.
"""

import os
from contextlib import ExitStack

import ml_dtypes
import numpy as np

import concourse.bass as bass
import concourse.bacc as bacc
import concourse.mybir as mybir
import concourse.tile as tile
from concourse.bass import ts
from concourse.bass_utils import run_bass_kernel_spmd

N_EXPERTS = 8
TOP_K = 2
D = 1024
N_CORES = 8
P = 128  # partitions
KT = D // P  # number of K tiles (8)
NH = 512  # psum free-dim tile (one bank of fp32)
G = int(os.environ.get("MOE_G", "2"))  # token tiles per DMA group

# matmul operand dtype: "bf16" (default) or "f32r" (TF32-class)
MM_DTYPE = os.environ.get("MOE_MM_DTYPE", "bf16")
# device output dtype: bf16 halves the output DMA; f32 for debugging
OUT_DTYPE = os.environ.get("MOE_OUT_DTYPE", "bf16")

LAST_RESULTS = None  # stash of BassKernelResults for test harness inspection

_BUILD_CACHE = {}


def _dt(name):
    return {
        "f32": mybir.dt.float32,
        "f32r": mybir.dt.float32r,
        "bf16": mybir.dt.bfloat16,
    }[name]


def _npdt(name):
    return {
        "f32": np.float32,
        "f32r": np.float32,
        "bf16": ml_dtypes.bfloat16,
    }[name]


def _build(C: int, repeat: int = 1, unroll: bool = False):
    """Build the SPMD Bass module for per-core padded token count C.

    Tiles 0..T-2 use weight slot `w`, tile T-1 uses slot `w2`.
    unroll=True emits the repeat loop as Python-unrolled copies (for
    TimelineSim, which cannot resolve the hardware-loop register branch).
    """
    key = (C, MM_DTYPE, OUT_DTYPE, repeat, G, unroll)
    if key in _BUILD_CACHE:
        return _BUILD_CACHE[key]

    f32 = mybir.dt.float32
    mm_dt = _dt(MM_DTYPE)
    out_dt = _dt(OUT_DTYPE)
    T = C // P

    nc = bacc.Bacc(None, target_bir_lowering=False)
    # xg_t: tiled gated tokens; xg_t[p, (t*KT + j)*P + c] = xg[t*128+c, j*128+p]
    xg_t = nc.declare_dram_parameter("xg_t", [P, T * KT * P], mm_dt, isOutput=False)
    w = nc.declare_dram_parameter("w", [D, D], mm_dt, isOutput=False)
    w2 = nc.declare_dram_parameter("w2", [D, D], mm_dt, isOutput=False)
    # y[p, t*D + f] = y_out[t*128 + p, f]
    y = nc.declare_dram_parameter("y", [P, T * D], out_dt, isOutput=True)

    with tile.TileContext(nc) as tc, ExitStack() as ctx:
        consts = ctx.enter_context(tc.tile_pool(name="consts", bufs=1))
        xpool = ctx.enter_context(tc.tile_pool(name="x", bufs=2))
        ypool = ctx.enter_context(tc.tile_pool(name="y", bufs=3))
        ypsum = ctx.enter_context(
            tc.tile_pool(name="ypsum", bufs=4, space=bass.MemorySpace.PSUM)
        )

        # ---- weights resident in SBUF ----
        w_sb = consts.tile([P, KT, D], mm_dt)
        nc.sync.dma_start(w_sb[:], w.rearrange("(kt p) n -> p kt n", p=P))
        w2_sb = consts.tile([P, KT, D], mm_dt)
        nc.sync.dma_start(w2_sb[:], w2.rearrange("(kt p) n -> p kt n", p=P))

        rep_cm = None
        if repeat > 1 and not unroll:
            rep_cm = tc.For_i(0, repeat, 1)
            rep_cm.__enter__()
        n_unroll = repeat if (repeat > 1 and unroll) else 1

        groups = [(g0, min(G, T - g0)) for g0 in range(0, T, G)]

        for _rep in range(n_unroll):
            # whole iteration's tokens in one DMA; bufs=2 overlaps the next
            # iteration's load with this iteration's compute
            xt = xpool.tile([P, T * KT * P], mm_dt, tag="xt")
            nc.sync.dma_start(xt[:], xg_t[:, :])
            for g0, gt in groups:
                ysb = ypool.tile([P, gt * D], out_dt, tag="ysb")
                for ti in range(gt):
                    tau = g0 + ti
                    wsel = w2_sb if tau == T - 1 else w_sb
                    yp = ypsum.tile([P, D], f32, tag="yp")
                    for nh in range(D // NH):
                        for j in range(KT):
                            nc.tensor.matmul(
                                yp[:, ts(nh, NH)],
                                xt[:, (tau * KT + j) * P : (tau * KT + j + 1) * P],
                                wsel[:, j, ts(nh, NH)],
                                start=(j == 0),
                                stop=(j == KT - 1),
                            )
                    dst = ysb[:, ti * D : (ti + 1) * D]
                    if tau % 2 == 0:
                        nc.vector.tensor_copy(dst, yp[:])
                    else:
                        nc.scalar.copy(dst, yp[:])
                # output DMA issued by the engine that wrote the last copy,
                # keeping the sync queue free for the big input loads
                nc.sync.dma_start(y[:, g0 * D : (g0 + gt) * D], ysb[:])

        if rep_cm is not None:
            rep_cm.__exit__(None, None, None)

    nc.compile()
    _BUILD_CACHE[key] = nc
    return nc


def _route(x, Wr, br):
    """Host routing in fp64: top-2 expert ids + top-2 softmax gates."""
    n_tokens = x.shape[0]
    logits = x.astype(np.float64) @ Wr.astype(np.float64) + br.astype(np.float64)
    i1 = np.argmax(logits, axis=1)
    l2 = logits.copy()
    l2[np.arange(n_tokens), i1] = -np.inf
    i2 = np.argmax(l2, axis=1)
    l1v = logits[np.arange(n_tokens), i1]
    l2v = logits[np.arange(n_tokens), i2]
    g1 = 1.0 / (1.0 + np.exp(l2v - l1v))
    g2 = 1.0 - g1
    return i1, i2, g1, g2


def _plan(idx_per_e, gate_per_e, n_tokens):
    """Balanced schedule: main tiles (own expert, capacity T0*128) + one
    overflow tile per core fed from experts whose load exceeds T0*128.

    Returns (T, mains, ovs): mains[e] = (idx, gates) len<=T0*128;
    ovs[core] = (expert, idx, gates) or None.
    """
    total = sum(len(ix) for ix in idx_per_e)
    T0 = -(-total // (P * N_CORES))
    while True:
        cap = T0 * P
        pieces = []
        for e in range(N_EXPERTS):
            ov = len(idx_per_e[e]) - cap
            o0 = cap
            while ov > 0:
                n = min(ov, P)
                pieces.append((e, o0, o0 + n))
                o0 += n
                ov -= n
        if len(pieces) <= N_CORES:
            break
        T0 += 1
    mains = [
        (idx_per_e[e][: T0 * P], gate_per_e[e][: T0 * P]) for e in range(N_EXPERTS)
    ]
    ovs = [None] * N_CORES
    for c, (e, a, bnd) in enumerate(pieces):
        ovs[c] = (e, idx_per_e[e][a:bnd], gate_per_e[e][a:bnd])
    has_ov = 1 if pieces else 0
    return T0 + has_ov, mains, ovs


def _pack_tokens(x, rows_idx, rows_gate, C):
    """Gated tokens -> [P, T*KT*P] partition-major tiled layout."""
    T = C // P
    xg = np.zeros((C, D), dtype=np.float32)
    n = len(rows_idx)
    if n:
        xg[:n] = x[rows_idx] * rows_gate[:, None].astype(np.float32)
    return np.ascontiguousarray(
        xg.reshape(T, P, KT, P).transpose(3, 0, 2, 1).reshape(P, T * KT * P)
    ).astype(_npdt(MM_DTYPE))


def _prep(inputs):
    x = np.asarray(inputs["x"], dtype=np.float32)
    Wr = np.asarray(inputs["Wr"], dtype=np.float32)
    br = np.asarray(inputs["br"], dtype=np.float32)
    W = np.asarray(inputs["W"], dtype=np.float32)
    b = np.asarray(inputs["b"], dtype=np.float32)
    i1, i2, g1, g2 = _route(x, Wr, br)
    idx_per_e, gate_per_e = [], []
    for e in range(N_EXPERTS):
        m1 = i1 == e
        m2 = i2 == e
        idx = np.concatenate([np.where(m1)[0], np.where(m2)[0]])
        gts = np.concatenate([g1[m1], g2[m2]])
        idx_per_e.append(idx)
        gate_per_e.append(gts)
    T, mains, ovs = _plan(idx_per_e, gate_per_e, x.shape[0])
    C = T * P
    T0 = T - 1

    np_mm = _npdt(MM_DTYPE)
    in_maps = []
    combine = []  # per core: (idx_main, g_main, e_main, ov or None)
    for e in range(N_CORES):
        m_idx, m_g = mains[e]
        # main tokens occupy tiles 0..T0-1; overflow tile T0 appended
        rows_idx = m_idx
        rows_gate = m_g
        pad_main = T0 * P - len(m_idx)
        ov = ovs[e]
        if ov is not None:
            o_e, o_idx, o_g = ov
            rows_idx = np.concatenate([m_idx, np.zeros(pad_main, np.int64), o_idx])
            rows_gate = np.concatenate([m_g, np.zeros(pad_main), o_g])
            # zero-gate the pad rows so x[0] garbage contributes nothing
            rows_gate[len(m_idx) : len(m_idx) + pad_main] = 0.0
            w2 = np.ascontiguousarray(W[o_e]).astype(np_mm)
        else:
            w2 = np.zeros((D, D), dtype=np_mm)
        xg_t = _pack_tokens(x, rows_idx, rows_gate, C)
        in_maps.append(
            {
                "xg_t": xg_t,
                "w": np.ascontiguousarray(W[e]).astype(np_mm),
                "w2": w2,
            }
        )
        combine.append((m_idx, m_g, e, ov))
    return in_maps, combine, C, x.shape[0], b


def kernel(**inputs) -> np.ndarray:
    global LAST_RESULTS
    in_maps, combine, C, n_tokens, b = _prep(inputs)
    T = C // P
    T0 = T - 1
    nc = _build(C)
    res = run_bass_kernel_spmd(nc, in_maps, core_ids=list(range(N_CORES)))
    LAST_RESULTS = res

    out = np.zeros((n_tokens, D), dtype=np.float32)
    for c in range(N_CORES):
        m_idx, m_g, e, ov = combine[c]
        # y [P, T*D]: y[p, t*D + f] = token (t*128+p), feature f
        ye = (
            np.asarray(res.results[c]["y"])
            .astype(np.float32)
            .reshape(P, T, D)
            .transpose(1, 0, 2)
            .reshape(C, D)
        )
        n = len(m_idx)
        # indices are unique within one expert's list -> fancy-index add is safe
        out[m_idx] += ye[:n] + m_g[:, None].astype(np.float32) * b[e]
        if ov is not None:
            o_e, o_idx, o_g = ov
            orows = ye[T0 * P : T0 * P + len(o_idx)]
            out[o_idx] += orows + o_g[:, None].astype(np.float32) * b[o_e]
    return out


# revision 9
# speedup vs baseline: 1.0955x; 1.0955x over previous
"""MoE (8 experts, top-2, d=1024, N=8192) on 8 trn2 NeuronCores.

Strategy (expert-parallel, per sharding hint):
 - Host computes routing (top-2 expert ids per token, fp64 logits) and the
   top-2 softmax gates, then dispatches WEIGHTED tokens: the gate is folded
   into the dispatched activation row (g*x), so the device kernel is a pure
   GEMM — "combine with weighted all-to-all back" done at dispatch time.
 - Load balancing: every core gets exactly T = T0+1 token tiles of 128.
   Tiles 0..T0-1 hold (up to) T0*128 tokens of the core's own expert
   (weight slot w). Tile T0 is an overflow tile: spill tokens from experts
   whose load exceeds T0*128 are redistributed across cores, each core
   receiving one <=128-token chunk with that expert's weights in slot w2.
   For the common near-balanced case this gives T=17 tiles/core instead of
   the 18 a max-load-padded per-expert split would need.
 - Device (per core, SPMD): for each 128-token tile: 16 matmuls (8 K-chunks
   x 2 psum halves of 512, j-outer so both halves share one LDWEIGHTS) in
   bf16 (1 cycle/row, half the DMA bytes of fp32), PSUM f32 accumulate,
   evacuate PSUM->SBUF with a bf16 downcast alternating Vector/Scalar
   engines, DMA out. Input tiles are DMA'd in groups of G for fewer, larger
   descriptors; groups are software-pipelined (load i+1 before compute i).
 - Host combines: out[idx] += y (+ g*b[e]); each token appears in exactly
   two experts' lists.
"""

import os
from contextlib import ExitStack

import ml_dtypes
import numpy as np

import concourse.bass as bass
import concourse.bacc as bacc
import concourse.mybir as mybir
import concourse.tile as tile
from concourse.bass import ts
from concourse.bass_utils import run_bass_kernel_spmd

N_EXPERTS = 8
TOP_K = 2
D = 1024
N_CORES = 8
P = 128  # partitions
KT = D // P  # number of K tiles (8)
NH = 512  # psum free-dim tile (one bank of fp32)
G = int(os.environ.get("MOE_G", "2"))  # token tiles per DMA group

# matmul operand dtype: "bf16" (default) or "f32r" (TF32-class)
MM_DTYPE = os.environ.get("MOE_MM_DTYPE", "bf16")
# device output dtype: bf16 halves the output DMA; f32 for debugging
OUT_DTYPE = os.environ.get("MOE_OUT_DTYPE", "bf16")

LAST_RESULTS = None  # stash of BassKernelResults for test harness inspection

_BUILD_CACHE = {}


def _dt(name):
    return {
        "f32": mybir.dt.float32,
        "f32r": mybir.dt.float32r,
        "bf16": mybir.dt.bfloat16,
    }[name]


def _npdt(name):
    return {
        "f32": np.float32,
        "f32r": np.float32,
        "bf16": ml_dtypes.bfloat16,
    }[name]


def _build(C: int, repeat: int = 1, unroll: bool = False):
    """Build the SPMD Bass module for per-core padded token count C.

    Tiles 0..T-2 use weight slot `w`, tile T-1 uses slot `w2`.
    unroll=True emits the repeat loop as Python-unrolled copies (for
    TimelineSim, which cannot resolve the hardware-loop register branch).
    """
    key = (C, MM_DTYPE, OUT_DTYPE, repeat, G, unroll)
    if key in _BUILD_CACHE:
        return _BUILD_CACHE[key]

    f32 = mybir.dt.float32
    mm_dt = _dt(MM_DTYPE)
    out_dt = _dt(OUT_DTYPE)
    T = C // P

    nc = bacc.Bacc(None, target_bir_lowering=False)
    # xg_t: tiled gated tokens; xg_t[p, (t*KT + j)*P + c] = xg[t*128+c, j*128+p]
    xg_t = nc.declare_dram_parameter("xg_t", [P, T * KT * P], mm_dt, isOutput=False)
    w = nc.declare_dram_parameter("w", [D, D], mm_dt, isOutput=False)
    w2 = nc.declare_dram_parameter("w2", [D, D], mm_dt, isOutput=False)
    # y[p, t*D + f] = y_out[t*128 + p, f]
    y = nc.declare_dram_parameter("y", [P, T * D], out_dt, isOutput=True)

    with tile.TileContext(nc) as tc, ExitStack() as ctx:
        consts = ctx.enter_context(tc.tile_pool(name="consts", bufs=1))
        xpool = ctx.enter_context(tc.tile_pool(name="x", bufs=3))
        ypool = ctx.enter_context(tc.tile_pool(name="y", bufs=3))
        ypsum = ctx.enter_context(
            tc.tile_pool(name="ypsum", bufs=4, space=bass.MemorySpace.PSUM)
        )

        # ---- weights resident in SBUF ----
        w_sb = consts.tile([P, KT, D], mm_dt)
        nc.sync.dma_start(w_sb[:], w.rearrange("(kt p) n -> p kt n", p=P))
        w2_sb = consts.tile([P, KT, D], mm_dt)
        nc.sync.dma_start(w2_sb[:], w2.rearrange("(kt p) n -> p kt n", p=P))

        rep_cm = None
        if repeat > 1 and not unroll:
            rep_cm = tc.For_i(0, repeat, 1)
            rep_cm.__enter__()
        n_unroll = repeat if (repeat > 1 and unroll) else 1

        groups = [(g0, min(G, T - g0)) for g0 in range(0, T, G)]

        def load_x(g0, gt):
            xt = xpool.tile([P, gt * KT * P], mm_dt, tag="xt")
            nc.sync.dma_start(xt[:], xg_t[:, g0 * KT * P : (g0 + gt) * KT * P])
            return xt

        def do_group(g0, gt, xt):
            ysb = ypool.tile([P, gt * D], out_dt, tag="ysb")
            for ti in range(gt):
                tau = g0 + ti
                wsel = w2_sb if tau == T - 1 else w_sb
                yp = ypsum.tile([P, D], f32, tag="yp")
                for nh in range(D // NH):
                    for j in range(KT):
                        nc.tensor.matmul(
                            yp[:, ts(nh, NH)],
                            xt[:, (ti * KT + j) * P : (ti * KT + j + 1) * P],
                            wsel[:, j, ts(nh, NH)],
                            start=(j == 0),
                            stop=(j == KT - 1),
                        )
                dst = ysb[:, ti * D : (ti + 1) * D]
                if tau % 2 == 0:
                    nc.vector.tensor_copy(dst, yp[:])
                else:
                    nc.scalar.copy(dst, yp[:])
            nc.sync.dma_start(y[:, g0 * D : (g0 + gt) * D], ysb[:])

        for _rep in range(n_unroll):
            xt_cur = load_x(*groups[0])
            for i, (g0, gt) in enumerate(groups):
                if i + 1 < len(groups):
                    xt_nxt = load_x(*groups[i + 1])
                do_group(g0, gt, xt_cur)
                if i + 1 < len(groups):
                    xt_cur = xt_nxt

        if rep_cm is not None:
            rep_cm.__exit__(None, None, None)

    nc.compile()
    _BUILD_CACHE[key] = nc
    return nc


def _route(x, Wr, br):
    """Host routing in fp64: top-2 expert ids + top-2 softmax gates."""
    n_tokens = x.shape[0]
    logits = x.astype(np.float64) @ Wr.astype(np.float64) + br.astype(np.float64)
    i1 = np.argmax(logits, axis=1)
    l2 = logits.copy()
    l2[np.arange(n_tokens), i1] = -np.inf
    i2 = np.argmax(l2, axis=1)
    l1v = logits[np.arange(n_tokens), i1]
    l2v = logits[np.arange(n_tokens), i2]
    g1 = 1.0 / (1.0 + np.exp(l2v - l1v))
    g2 = 1.0 - g1
    return i1, i2, g1, g2


def _plan(idx_per_e, gate_per_e, n_tokens):
    """Balanced schedule: main tiles (own expert, capacity T0*128) + one
    overflow tile per core fed from experts whose load exceeds T0*128.

    Returns (T, mains, ovs): mains[e] = (idx, gates) len<=T0*128;
    ovs[core] = (expert, idx, gates) or None.
    """
    total = sum(len(ix) for ix in idx_per_e)
    T0 = -(-total // (P * N_CORES))
    while True:
        cap = T0 * P
        pieces = []
        for e in range(N_EXPERTS):
            ov = len(idx_per_e[e]) - cap
            o0 = cap
            while ov > 0:
                n = min(ov, P)
                pieces.append((e, o0, o0 + n))
                o0 += n
                ov -= n
        if len(pieces) <= N_CORES:
            break
        T0 += 1
    mains = [
        (idx_per_e[e][: T0 * P], gate_per_e[e][: T0 * P]) for e in range(N_EXPERTS)
    ]
    ovs = [None] * N_CORES
    for c, (e, a, bnd) in enumerate(pieces):
        ovs[c] = (e, idx_per_e[e][a:bnd], gate_per_e[e][a:bnd])
    has_ov = 1 if pieces else 0
    return T0 + has_ov, mains, ovs


def _pack_tokens(x, rows_idx, rows_gate, C):
    """Gated tokens -> [P, T*KT*P] partition-major tiled layout."""
    T = C // P
    xg = np.zeros((C, D), dtype=np.float32)
    n = len(rows_idx)
    if n:
        xg[:n] = x[rows_idx] * rows_gate[:, None].astype(np.float32)
    return np.ascontiguousarray(
        xg.reshape(T, P, KT, P).transpose(3, 0, 2, 1).reshape(P, T * KT * P)
    ).astype(_npdt(MM_DTYPE))


def _prep(inputs):
    x = np.asarray(inputs["x"], dtype=np.float32)
    Wr = np.asarray(inputs["Wr"], dtype=np.float32)
    br = np.asarray(inputs["br"], dtype=np.float32)
    W = np.asarray(inputs["W"], dtype=np.float32)
    b = np.asarray(inputs["b"], dtype=np.float32)
    i1, i2, g1, g2 = _route(x, Wr, br)
    idx_per_e, gate_per_e = [], []
    for e in range(N_EXPERTS):
        m1 = i1 == e
        m2 = i2 == e
        idx = np.concatenate([np.where(m1)[0], np.where(m2)[0]])
        gts = np.concatenate([g1[m1], g2[m2]])
        idx_per_e.append(idx)
        gate_per_e.append(gts)
    T, mains, ovs = _plan(idx_per_e, gate_per_e, x.shape[0])
    C = T * P
    T0 = T - 1

    np_mm = _npdt(MM_DTYPE)
    in_maps = []
    combine = []  # per core: (idx_main, g_main, e_main, ov or None)
    for e in range(N_CORES):
        m_idx, m_g = mains[e]
        # main tokens occupy tiles 0..T0-1; overflow tile T0 appended
        rows_idx = m_idx
        rows_gate = m_g
        pad_main = T0 * P - len(m_idx)
        ov = ovs[e]
        if ov is not None:
            o_e, o_idx, o_g = ov
            rows_idx = np.concatenate([m_idx, np.zeros(pad_main, np.int64), o_idx])
            rows_gate = np.concatenate([m_g, np.zeros(pad_main), o_g])
            # zero-gate the pad rows so x[0] garbage contributes nothing
            rows_gate[len(m_idx) : len(m_idx) + pad_main] = 0.0
            w2 = np.ascontiguousarray(W[o_e]).astype(np_mm)
        else:
            w2 = np.zeros((D, D), dtype=np_mm)
        xg_t = _pack_tokens(x, rows_idx, rows_gate, C)
        in_maps.append(
            {
                "xg_t": xg_t,
                "w": np.ascontiguousarray(W[e]).astype(np_mm),
                "w2": w2,
            }
        )
        combine.append((m_idx, m_g, e, ov))
    return in_maps, combine, C, x.shape[0], b


def kernel(**inputs) -> np.ndarray:
    global LAST_RESULTS
    in_maps, combine, C, n_tokens, b = _prep(inputs)
    T = C // P
    T0 = T - 1
    nc = _build(C)
    res = run_bass_kernel_spmd(nc, in_maps, core_ids=list(range(N_CORES)))
    LAST_RESULTS = res

    out = np.zeros((n_tokens, D), dtype=np.float32)
    for c in range(N_CORES):
        m_idx, m_g, e, ov = combine[c]
        # y [P, T*D]: y[p, t*D + f] = token (t*128+p), feature f
        ye = (
            np.asarray(res.results[c]["y"])
            .astype(np.float32)
            .reshape(P, T, D)
            .transpose(1, 0, 2)
            .reshape(C, D)
        )
        n = len(m_idx)
        # indices are unique within one expert's list -> fancy-index add is safe
        out[m_idx] += ye[:n] + m_g[:, None].astype(np.float32) * b[e]
        if ov is not None:
            o_e, o_idx, o_g = ov
            orows = ye[T0 * P : T0 * P + len(o_idx)]
            out[o_idx] += orows + o_g[:, None].astype(np.float32) * b[o_e]
    return out


# revision 11
# speedup vs baseline: 1.1121x; 1.0152x over previous
"""MoE (8 experts, top-2, d=1024, N=8192) on 8 trn2 NeuronCores.

Strategy (expert-parallel, per sharding hint):
 - Host computes routing (top-2 expert ids per token, fp64 logits) and the
   top-2 softmax gates, then dispatches WEIGHTED tokens: the gate is folded
   into the dispatched activation row (g*x), so the device kernel is a pure
   GEMM — "combine with weighted all-to-all back" done at dispatch time.
 - Load balancing: every core gets exactly T = T0+1 token tiles of 128.
   Tiles 0..T0-1 hold (up to) T0*128 tokens of the core's own expert
   (weight slot w). Tile T0 is an overflow tile: spill tokens from experts
   whose load exceeds T0*128 are redistributed across cores, each core
   receiving one <=128-token chunk with that expert's weights in slot w2.
   For the common near-balanced case this gives T=17 tiles/core instead of
   the 18 a max-load-padded per-expert split would need.
 - Device (per core, SPMD): for each 128-token tile: 16 matmuls (8 K-chunks
   x 2 psum halves of 512, j-outer so both halves share one LDWEIGHTS) in
   bf16 (1 cycle/row, half the DMA bytes of fp32), PSUM f32 accumulate,
   evacuate PSUM->SBUF with a bf16 downcast alternating Vector/Scalar
   engines, DMA out. Input tiles are DMA'd in groups of G for fewer, larger
   descriptors; groups are software-pipelined (load i+1 before compute i).
 - Host combines: out[idx] += y (+ g*b[e]); each token appears in exactly
   two experts' lists.
"""

import os
from contextlib import ExitStack

import ml_dtypes
import numpy as np

import concourse.bass as bass
import concourse.bacc as bacc
import concourse.mybir as mybir
import concourse.tile as tile
from concourse.bass import ts
from concourse.bass_utils import run_bass_kernel_spmd

N_EXPERTS = 8
TOP_K = 2
D = 1024
N_CORES = 8
P = 128  # partitions
KT = D // P  # number of K tiles (8)
NH = 512  # psum free-dim tile (one bank of fp32)
G = int(os.environ.get("MOE_G", "2"))  # token tiles per DMA group

# matmul operand dtype: "bf16" (default) or "f32r" (TF32-class)
MM_DTYPE = os.environ.get("MOE_MM_DTYPE", "bf16")
# device output dtype: bf16 halves the output DMA; f32 for debugging
OUT_DTYPE = os.environ.get("MOE_OUT_DTYPE", "bf16")
# psum->sbuf copy engines: "alt" (vector/scalar) or "dve" (vector only)
COPY_ENG = os.environ.get("MOE_COPY", "alt")
# output-DMA issue queue: "sp" (sync) or "act" (scalar/Activation HWDGE)
OUT_Q = os.environ.get("MOE_OUTQ", "sp")

LAST_RESULTS = None  # stash of BassKernelResults for test harness inspection

_BUILD_CACHE = {}


def _dt(name):
    return {
        "f32": mybir.dt.float32,
        "f32r": mybir.dt.float32r,
        "bf16": mybir.dt.bfloat16,
    }[name]


def _npdt(name):
    return {
        "f32": np.float32,
        "f32r": np.float32,
        "bf16": ml_dtypes.bfloat16,
    }[name]


def _build(C: int, repeat: int = 1, unroll: bool = False):
    """Build the SPMD Bass module for per-core padded token count C.

    Tiles 0..T-2 use weight slot `w`, tile T-1 uses slot `w2`.
    unroll=True emits the repeat loop as Python-unrolled copies (for
    TimelineSim, which cannot resolve the hardware-loop register branch).
    """
    key = (C, MM_DTYPE, OUT_DTYPE, repeat, G, unroll, COPY_ENG, OUT_Q)
    if key in _BUILD_CACHE:
        return _BUILD_CACHE[key]

    f32 = mybir.dt.float32
    mm_dt = _dt(MM_DTYPE)
    out_dt = _dt(OUT_DTYPE)
    T = C // P

    nc = bacc.Bacc(None, target_bir_lowering=False)
    # xg_t: tiled gated tokens; xg_t[p, (t*KT + j)*P + c] = xg[t*128+c, j*128+p]
    xg_t = nc.declare_dram_parameter("xg_t", [P, T * KT * P], mm_dt, isOutput=False)
    w = nc.declare_dram_parameter("w", [D, D], mm_dt, isOutput=False)
    w2 = nc.declare_dram_parameter("w2", [D, D], mm_dt, isOutput=False)
    # y[p, t*D + f] = y_out[t*128 + p, f]
    y = nc.declare_dram_parameter("y", [P, T * D], out_dt, isOutput=True)

    with tile.TileContext(nc) as tc, ExitStack() as ctx:
        consts = ctx.enter_context(tc.tile_pool(name="consts", bufs=1))
        xpool = ctx.enter_context(tc.tile_pool(name="x", bufs=3))
        ypool = ctx.enter_context(tc.tile_pool(name="y", bufs=3))
        ypsum = ctx.enter_context(
            tc.tile_pool(name="ypsum", bufs=4, space=bass.MemorySpace.PSUM)
        )

        # ---- weights resident in SBUF ----
        w_sb = consts.tile([P, KT, D], mm_dt)
        nc.sync.dma_start(w_sb[:], w.rearrange("(kt p) n -> p kt n", p=P))
        w2_sb = consts.tile([P, KT, D], mm_dt)
        nc.sync.dma_start(w2_sb[:], w2.rearrange("(kt p) n -> p kt n", p=P))

        rep_cm = None
        if repeat > 1 and not unroll:
            rep_cm = tc.For_i(0, repeat, 1)
            rep_cm.__enter__()
        n_unroll = repeat if (repeat > 1 and unroll) else 1

        groups = [(g0, min(G, T - g0)) for g0 in range(0, T, G)]

        def load_x(g0, gt):
            xt = xpool.tile([P, gt * KT * P], mm_dt, tag="xt")
            nc.sync.dma_start(xt[:], xg_t[:, g0 * KT * P : (g0 + gt) * KT * P])
            return xt

        def do_group(g0, gt, xt):
            ysb = ypool.tile([P, gt * D], out_dt, tag="ysb")
            for ti in range(gt):
                tau = g0 + ti
                wsel = w2_sb if tau == T - 1 else w_sb
                yp = ypsum.tile([P, D], f32, tag="yp")
                for nh in range(D // NH):
                    for j in range(KT):
                        nc.tensor.matmul(
                            yp[:, ts(nh, NH)],
                            xt[:, (ti * KT + j) * P : (ti * KT + j + 1) * P],
                            wsel[:, j, ts(nh, NH)],
                            start=(j == 0),
                            stop=(j == KT - 1),
                        )
                dst = ysb[:, ti * D : (ti + 1) * D]
                if COPY_ENG == "dve" or tau % 2 == 0:
                    nc.vector.tensor_copy(dst, yp[:])
                else:
                    nc.scalar.copy(dst, yp[:])
            if OUT_Q == "act":
                nc.scalar.dma_start(y[:, g0 * D : (g0 + gt) * D], ysb[:])
            else:
                nc.sync.dma_start(y[:, g0 * D : (g0 + gt) * D], ysb[:])

        for _rep in range(n_unroll):
            xt_cur = load_x(*groups[0])
            for i, (g0, gt) in enumerate(groups):
                if i + 1 < len(groups):
                    xt_nxt = load_x(*groups[i + 1])
                do_group(g0, gt, xt_cur)
                if i + 1 < len(groups):
                    xt_cur = xt_nxt

        if rep_cm is not None:
            rep_cm.__exit__(None, None, None)

    nc.compile()
    _BUILD_CACHE[key] = nc
    return nc


def _route(x, Wr, br):
    """Host routing in fp64: top-2 expert ids + top-2 softmax gates."""
    n_tokens = x.shape[0]
    logits = x.astype(np.float64) @ Wr.astype(np.float64) + br.astype(np.float64)
    i1 = np.argmax(logits, axis=1)
    l2 = logits.copy()
    l2[np.arange(n_tokens), i1] = -np.inf
    i2 = np.argmax(l2, axis=1)
    l1v = logits[np.arange(n_tokens), i1]
    l2v = logits[np.arange(n_tokens), i2]
    g1 = 1.0 / (1.0 + np.exp(l2v - l1v))
    g2 = 1.0 - g1
    return i1, i2, g1, g2


def _plan(idx_per_e, gate_per_e, n_tokens):
    """Balanced schedule: main tiles (own expert, capacity T0*128) + one
    overflow tile per core fed from experts whose load exceeds T0*128.

    Returns (T, mains, ovs): mains[e] = (idx, gates) len<=T0*128;
    ovs[core] = (expert, idx, gates) or None.
    """
    total = sum(len(ix) for ix in idx_per_e)
    T0 = -(-total // (P * N_CORES))
    while True:
        cap = T0 * P
        pieces = []
        for e in range(N_EXPERTS):
            ov = len(idx_per_e[e]) - cap
            o0 = cap
            while ov > 0:
                n = min(ov, P)
                pieces.append((e, o0, o0 + n))
                o0 += n
                ov -= n
        if len(pieces) <= N_CORES:
            break
        T0 += 1
    mains = [
        (idx_per_e[e][: T0 * P], gate_per_e[e][: T0 * P]) for e in range(N_EXPERTS)
    ]
    ovs = [None] * N_CORES
    for c, (e, a, bnd) in enumerate(pieces):
        ovs[c] = (e, idx_per_e[e][a:bnd], gate_per_e[e][a:bnd])
    has_ov = 1 if pieces else 0
    return T0 + has_ov, mains, ovs


def _pack_tokens(x, rows_idx, rows_gate, C):
    """Gated tokens -> [P, T*KT*P] partition-major tiled layout."""
    T = C // P
    xg = np.zeros((C, D), dtype=np.float32)
    n = len(rows_idx)
    if n:
        xg[:n] = x[rows_idx] * rows_gate[:, None].astype(np.float32)
    return np.ascontiguousarray(
        xg.reshape(T, P, KT, P).transpose(3, 0, 2, 1).reshape(P, T * KT * P)
    ).astype(_npdt(MM_DTYPE))


def _prep(inputs):
    x = np.asarray(inputs["x"], dtype=np.float32)
    Wr = np.asarray(inputs["Wr"], dtype=np.float32)
    br = np.asarray(inputs["br"], dtype=np.float32)
    W = np.asarray(inputs["W"], dtype=np.float32)
    b = np.asarray(inputs["b"], dtype=np.float32)
    i1, i2, g1, g2 = _route(x, Wr, br)
    idx_per_e, gate_per_e = [], []
    for e in range(N_EXPERTS):
        m1 = i1 == e
        m2 = i2 == e
        idx = np.concatenate([np.where(m1)[0], np.where(m2)[0]])
        gts = np.concatenate([g1[m1], g2[m2]])
        idx_per_e.append(idx)
        gate_per_e.append(gts)
    T, mains, ovs = _plan(idx_per_e, gate_per_e, x.shape[0])
    C = T * P
    T0 = T - 1

    np_mm = _npdt(MM_DTYPE)
    in_maps = []
    combine = []  # per core: (idx_main, g_main, e_main, ov or None)
    for e in range(N_CORES):
        m_idx, m_g = mains[e]
        # main tokens occupy tiles 0..T0-1; overflow tile T0 appended
        rows_idx = m_idx
        rows_gate = m_g
        pad_main = T0 * P - len(m_idx)
        ov = ovs[e]
        if ov is not None:
            o_e, o_idx, o_g = ov
            rows_idx = np.concatenate([m_idx, np.zeros(pad_main, np.int64), o_idx])
            rows_gate = np.concatenate([m_g, np.zeros(pad_main), o_g])
            # zero-gate the pad rows so x[0] garbage contributes nothing
            rows_gate[len(m_idx) : len(m_idx) + pad_main] = 0.0
            w2 = np.ascontiguousarray(W[o_e]).astype(np_mm)
        else:
            w2 = np.zeros((D, D), dtype=np_mm)
        xg_t = _pack_tokens(x, rows_idx, rows_gate, C)
        in_maps.append(
            {
                "xg_t": xg_t,
                "w": np.ascontiguousarray(W[e]).astype(np_mm),
                "w2": w2,
            }
        )
        combine.append((m_idx, m_g, e, ov))
    return in_maps, combine, C, x.shape[0], b


def kernel(**inputs) -> np.ndarray:
    global LAST_RESULTS
    in_maps, combine, C, n_tokens, b = _prep(inputs)
    T = C // P
    T0 = T - 1
    nc = _build(C)
    res = run_bass_kernel_spmd(nc, in_maps, core_ids=list(range(N_CORES)))
    LAST_RESULTS = res

    out = np.zeros((n_tokens, D), dtype=np.float32)
    for c in range(N_CORES):
        m_idx, m_g, e, ov = combine[c]
        # y [P, T*D]: y[p, t*D + f] = token (t*128+p), feature f
        ye = (
            np.asarray(res.results[c]["y"])
            .astype(np.float32)
            .reshape(P, T, D)
            .transpose(1, 0, 2)
            .reshape(C, D)
        )
        n = len(m_idx)
        # indices are unique within one expert's list -> fancy-index add is safe
        out[m_idx] += ye[:n] + m_g[:, None].astype(np.float32) * b[e]
        if ov is not None:
            o_e, o_idx, o_g = ov
            orows = ye[T0 * P : T0 * P + len(o_idx)]
            out[o_idx] += orows + o_g[:, None].astype(np.float32) * b[o_e]
    return out
